# revision 51
# baseline (speedup 1.0000x reference)
"""Fused LayerNorm + causal multi-head attention (with additive bias) + out-proj
for Trainium2, SPMD over 8 NeuronCores.

Sharding: tensor-parallel over heads. 16 heads / 8 cores = 2 heads per core.
Each core computes LN(x) (replicated), the qkv projection restricted to its
2 heads' columns, causal softmax attention with its heads' bias slices, and a
partial output projection (its heads' rows of w_out). Host sums the 8 partial
outputs (the TP all-reduce, done on gather).

v2 layout/schedule choices (on top of the v1 algebra):
 - All weight prep is host-side: wqkvb ships bf16, gamma-folded and q-scaled;
   the rank-1 LN correction rows (-colsum(W'), beta@W) ship precomputed, and
   both rank-1 terms ride ONE K=2 matmul against a stacked (mu, sd) row pair.
 - x ships pre-sliced per (batch, 512-token slice) so the first stats matmuls
   start ~3us in and the PE never goes HAM-cold waiting on the full x load.
   Batches interleave (nt outer, b inner) so attention i-tile 0 is unblocked
   after two slice iterations.
 - Scores are computed transposed, per (i-tile, batch) pass with the two
   heads' score blocks in one 2-bank PSUM tile: the two score matmuls are
   K=64 row-tiled (h0 rows 0-63, h1 rows 64-127) and run concurrently, and
   ONE exp activation covers both heads (halves ACT op overhead).
 - P@V trails the score/bias matmuls by one j-block (software pipeline) so
   the PE never waits on the exp.
 - The out-projection for i-tile t is emitted right after the tile's attn
   pass: the PE stays dense to the end (no HAM re-throttle at the tail) and
   y (bf16) streams out early.
"""

import numpy as np
import ml_dtypes
from contextlib import ExitStack

import concourse.bass as bass
import concourse.tile as tile
from concourse import bacc, mybir
from concourse.bass_utils import run_bass_kernel_spmd

F32 = mybir.dt.float32
BF16 = mybir.dt.bfloat16
AL = mybir.AluOpType
ACTF = mybir.ActivationFunctionType

N_CORES = 8
B = 2            # batch
N = 2048         # tokens
D = 1024         # model dim
H = 16           # total heads
HL = 2           # heads per core
DH = 64          # head dim
COLS = 3 * HL * DH   # 384 qkv columns per core
KS = D // 128    # 8 contraction slabs
TT = N // 128    # 16 token tiles
IT = N // 512    # 4 i-tiles (query tiles of 512)
SCALE = DH ** -0.5
LN_EPS = 1e-5
NEG = -1.0e9

MM = dict(skip_group_check=True)


def build_program(has_beta=False):
    nc = bacc.Bacc("TRN2", target_bir_lowering=False, debug=False)

    xs_in = nc.dram_tensor("xs", [B, IT, 128, KS * 512], BF16, kind="ExternalInput")
    biasT_in = nc.dram_tensor("biasT", [HL, N, N], BF16, kind="ExternalInput")
    wqkv_in = nc.dram_tensor("wqkvb", [128, KS, COLS], BF16, kind="ExternalInput")
    bwun_in = nc.dram_tensor("bwun", [2, COLS], BF16, kind="ExternalInput")
    wout_in = nc.dram_tensor("woutb", [128, D], BF16, kind="ExternalInput")
    ident_in = nc.dram_tensor("ident", [128, 128], BF16, kind="ExternalInput")
    y_out = nc.dram_tensor("y", [B, N, D], BF16, kind="ExternalOutput")

    with tile.TileContext(nc) as tc, ExitStack() as ctx:
        # ---- persistent sbuf ----
        pers = ctx.enter_context(tc.tile_pool(name="pers", bufs=1))
        qT = [pers.tile([128, N], BF16, tag=f"qT{b}", name=f"qT{b}") for b in range(B)]
        kT = [pers.tile([128, N], BF16, tag=f"kT{b}", name=f"kT{b}") for b in range(B)]
        vT = [pers.tile([128, N], BF16, tag=f"vT{b}", name=f"vT{b}") for b in range(B)]
        # V natural with ones column: per key-tile [.., 130]: h0 v(64)+1, h1 v(64)+1
        vA = [pers.tile([128, TT, 130], BF16, tag=f"vA{b}", name=f"vA{b}") for b in range(B)]
        oT = [pers.tile([128, N], BF16, tag=f"oT{b}", name=f"oT{b}") for b in range(B)]
        # LN correction rows: mu per token (and sd when beta != 0)
        s_bf = [pers.tile([1, N], BF16, tag=f"sb{b}", name=f"sb{b}") for b in range(B)]
        sd_bf = [pers.tile([1, N], BF16, tag=f"sd{b}", name=f"sd{b}")
                 for b in range(B)] if has_beta else None
        wqb = pers.tile([128, KS, COLS], BF16, tag="wqb")
        bwun = pers.tile([2, COLS], BF16, tag="bwun")
        wob = pers.tile([128, D], BF16, tag="wob")
        ident = pers.tile([128, 128], BF16, tag="ident")
        onesd = pers.tile([128, 1], BF16, tag="onesd")    # 1/D for stats matmuls
        epsc = pers.tile([128, 1], F32, tag="epsc")
        zeroc = pers.tile([128, 1], F32, tag="zeroc")

        nc.vector.memset(onesd[:], 1.0 / D)
        nc.vector.memset(epsc[:], LN_EPS)
        nc.vector.memset(zeroc[:], 0.0)
        for b in range(B):
            nc.vector.memset(
                vA[b][:, :, 64::65].rearrange("p t o -> p (t o)"), 1.0)

        nc.sync.dma_start(wqb[:], wqkv_in.ap())
        nc.sync.dma_start(bwun[:], bwun_in.ap())
        nc.sync.dma_start(wob[:], wout_in.ap())
        nc.sync.dma_start(ident[:], ident_in.ap())

        # ---- LN stats + qkv^T, pipelined per (512-token slice, batch) ----
        xpool = ctx.enter_context(tc.tile_pool(name="xs", bufs=4))
        x2pool = ctx.enter_context(tc.tile_pool(name="x2", bufs=2))
        rows = ctx.enter_context(tc.tile_pool(name="rows", bufs=3))
        rbc = ctx.enter_context(tc.tile_pool(name="rbc", bufs=3))
        with tc.tile_pool(name="pstat", bufs=2, space="PSUM") as pstat, \
             tc.tile_pool(name="pqkv", bufs=3, space="PSUM") as pqkv, \
             tc.tile_pool(name="pvt", bufs=1, space="PSUM") as pvt:
            dsts = (qT, kT, vT)
            for nt in range(IT):
                sl = slice(nt * 512, (nt + 1) * 512)
                for b in range(B):
                    xk = xpool.tile([128, KS, 512], BF16, tag="xk",
                                    name=f"xk{b}_{nt}")
                    if nt == 0:
                        # per-slab loads: the first stats matmul starts after
                        # 128KB instead of 1MB
                        for k in range(KS):
                            nc.sync.dma_start(
                                xk[:, k],
                                xs_in.ap()[b, nt, :, k * 512:(k + 1) * 512])
                    else:
                        nc.sync.dma_start(
                            xk[:].rearrange("p k i -> p (k i)"), xs_in.ap()[b, nt])
                    # stats: slab-sum of x on DVE (bf16 tree), then one matmul
                    # for mu; E[x^2] via squared moving stays on the PE
                    xsum = x2pool.tile([128, 4, 512], BF16, tag="xsum")
                    nc.vector.tensor_tensor(
                        xsum[:].rearrange("p k i -> p (k i)"),
                        xk[:, 0:4].rearrange("p k i -> p (k i)"),
                        xk[:, 4:8].rearrange("p k i -> p (k i)"), op=AL.add)
                    nc.vector.tensor_tensor(
                        xsum[:, 0:2].rearrange("p k i -> p (k i)"),
                        xsum[:, 0:2].rearrange("p k i -> p (k i)"),
                        xsum[:, 2:4].rearrange("p k i -> p (k i)"), op=AL.add)
                    nc.vector.tensor_tensor(xsum[:, 0], xsum[:, 0], xsum[:, 1],
                                            op=AL.add)
                    mu_ps = pstat.tile([1, 512], F32, tag="mu_ps")
                    nc.tensor.matmul(mu_ps[:], onesd[:], xsum[:, 0],
                                     start=True, stop=True, **MM)
                    # squares split across the scalar + vector engines
                    x2 = x2pool.tile([128, KS, 512], BF16, tag="x2")
                    nc.scalar.activation(
                        x2[:, 0:4].rearrange("p k i -> p (k i)"),
                        xk[:, 0:4].rearrange("p k i -> p (k i)"),
                        ACTF.Square, bias=zeroc[:])
                    nc.vector.tensor_tensor(
                        x2[:, 4:8].rearrange("p k i -> p (k i)"),
                        xk[:, 4:8].rearrange("p k i -> p (k i)"),
                        xk[:, 4:8].rearrange("p k i -> p (k i)"), op=AL.mult)
                    x2_ps = pstat.tile([1, 512], F32, tag="x2_ps")
                    for k in range(KS):
                        nc.tensor.matmul(x2_ps[:], onesd[:], x2[:, k],
                                         start=(k == 0), stop=(k == KS - 1), **MM)
                    # row chain: var = E[x^2]-mu^2, rsig = rsqrt(var+eps)
                    var_r = rows.tile([1, 512], F32, tag="var_r")
                    nc.scalar.activation(var_r[:], mu_ps[:], ACTF.Square,
                                         bias=zeroc[0:1, :])
                    nc.vector.tensor_tensor(var_r[:], x2_ps[:], var_r[:],
                                            op=AL.subtract)
                    sd_r = rows.tile([1, 512], F32, tag="sd_r")
                    nc.scalar.activation(sd_r[:], var_r[:], ACTF.Sqrt,
                                         bias=epsc[0:1, :])
                    rsig_r = rows.tile([1, 512], F32, tag="rsig_r")
                    nc.vector.reciprocal_approx_fast(rsig_r[:], sd_r[:])
                    # bf16 rows feed the rank-1 correction matmuls
                    nc.scalar.copy(s_bf[b][0:1, sl], mu_ps[:])
                    if has_beta:
                        nc.vector.tensor_copy(sd_bf[b][0:1, sl], sd_r[:])
                    rsig_bc = rbc.tile([128, 512], F32, tag="rsig_bc")
                    nc.gpsimd.partition_broadcast(rsig_bc[:], rsig_r[:],
                                                  channels=128)
                    # qkv projection (raw), rank-1 LN corrections fused in PSUM,
                    # per-token rsig rides the eviction
                    for blk in range(3):
                        csl = slice(blk * 128, (blk + 1) * 128)
                        ps = pqkv.tile([128, 512], F32, tag="psqkv")
                        for k in range(KS):
                            nc.tensor.matmul(ps[:], wqb[:, k, csl], xk[:, k],
                                             start=(k == 0), stop=False, **MM)
                        nc.tensor.matmul(ps[:], bwun[0:1, csl], s_bf[b][:, sl],
                                         start=False, stop=(not has_beta), **MM)
                        if has_beta:
                            nc.tensor.matmul(ps[:], bwun[1:2, csl], sd_bf[b][:, sl],
                                             start=False, stop=True, **MM)
                        nc.vector.tensor_tensor(dsts[blk][b][:, sl], ps[:],
                                                rsig_bc[:], op=AL.mult)
                    # V -> natural layout (ones column pre-memset)
                    for t in range(4 * nt, 4 * nt + 4):
                        pst = pvt.tile([128, 128], BF16, tag="pst")
                        nc.tensor.transpose(
                            pst[:], vT[b][:, t * 128:(t + 1) * 128], ident[:])
                        nc.scalar.copy(
                            vA[b][:, t, :].rearrange("p (h v) -> p h v", h=2)[:, :, 0:64],
                            pst[:].rearrange("p (h v) -> p h v", h=2))

        # ---- attention + out-projection, interleaved per i-tile ----
        with tc.tile_pool(name="bias", bufs=34) as bias_pool, \
             tc.tile_pool(name="pexp", bufs=6) as exp_pool, \
             tc.tile_pool(name="lnrm", bufs=4) as lnrm, \
             tc.tile_pool(name="pss", bufs=3, space="PSUM") as pss_pool, \
             tc.tile_pool(name="pso", bufs=1, space="PSUM") as pso_pool:
            for t in range(IT):
                isl = slice(t * 512, (t + 1) * 512)
                nj = 4 * (t + 1)
                # prefetch the whole i-tile's bias blocks ahead of the j-loops
                bts = {}
                for j in range(nj):
                    off = max(0, 128 * j - 512 * t)
                    islo = slice(t * 512 + off, (t + 1) * 512)
                    for h in range(HL):
                        bt = bias_pool.tile([128, 512], BF16, tag="bt")
                        nc.sync.dma_start(
                            bt[:, off:],
                            biasT_in.ap()[h, j * 128:(j + 1) * 128, islo])
                        bts[(j, h)] = bt
                for b in range(B):
                    pso = {h: pso_pool.tile([65, 512], F32, tag=f"pso{h}",
                                            name=f"pso{t}{b}{h}")
                           for h in range(HL)}
                    pend = []  # [(j, off, pe)] awaiting P@V — two blocks behind
                    for j in range(nj):
                        # columns i < 128j of this i-slice are fully masked:
                        # skip them in every op (causal trim)
                        off = max(0, 128 * j - 512 * t)
                        islo = slice(t * 512 + off, (t + 1) * 512)
                        ps = pss_pool.tile([128, 2, 512], F32, tag="pss")
                        for h in range(HL):
                            hsl = slice(h * 64, (h + 1) * 64)
                            nc.tensor.matmul(
                                ps[:, h, off:], kT[b][hsl, j * 128:(j + 1) * 128],
                                qT[b][hsl, islo],
                                start=True, stop=(h == 0), **MM)
                        # bias add split across engines: h0 on DVE (idle during
                        # the j-loop), h1 as a PE identity-matmul accumulate
                        nc.tensor.matmul(ps[:, 1, off:], ident[:],
                                         bts[(j, 1)][:, off:],
                                         start=False, stop=True, **MM)
                        nc.vector.tensor_tensor(ps[:, 0, off:], ps[:, 0, off:],
                                                bts[(j, 0)][:, off:], op=AL.add)
                        if len(pend) >= 2:
                            pj, poff, ppe = pend.pop(0)
                            for h in range(HL):
                                nc.tensor.matmul(
                                    pso[h][:, poff:],
                                    vA[b][:, pj, h * 65:h * 65 + 65],
                                    ppe[:, h, poff:],
                                    start=(pj == 0), stop=False, **MM)
                        pe = exp_pool.tile([128, 2, 512], BF16, tag="pe")
                        nc.scalar.activation(pe[:, :, off:], ps[:, :, off:],
                                             ACTF.Exp, bias=zeroc[:])
                        pend.append((j, off, pe))
                    for pj, poff, ppe in pend:
                        for h in range(HL):
                            nc.tensor.matmul(
                                pso[h][:, poff:], vA[b][:, pj, h * 65:h * 65 + 65],
                                ppe[:, h, poff:],
                                start=(pj == 0), stop=(pj == nj - 1), **MM)
                    # deferred softmax normalization: 1/l from the ones column
                    for h in range(HL):
                        lrow = lnrm.tile([1, 512], F32, tag="lrow")
                        nc.vector.tensor_copy(lrow[:], pso[h][64:65, :])
                        rec = lnrm.tile([1, 512], F32, tag="rec")
                        nc.vector.reciprocal_approx_fast(rec[:], lrow[:])
                        lb = lnrm.tile([64, 512], F32, tag="lb")
                        nc.gpsimd.partition_broadcast(lb[:], rec[:], channels=64)
                        nc.vector.tensor_tensor(
                            oT[b][h * 64:(h + 1) * 64, isl],
                            pso[h][0:64, :], lb[:], op=AL.mult)
        # out-projection: one dense matmul block (PE stays HAM-warm through it)
        with tc.tile_pool(name="ysb", bufs=8) as ysb, \
             tc.tile_pool(name="py", bufs=4, space="PSUM") as py_pool:
            for b in range(B):
                for tt in range(TT):
                    psy = py_pool.tile([128, 2, 512], F32, tag="psy")
                    for half in range(2):
                        nc.tensor.matmul(psy[:, half],
                                         oT[b][:, tt * 128:(tt + 1) * 128],
                                         wob[:, half * 512:(half + 1) * 512],
                                         start=True, stop=True, **MM)
                    yt = ysb.tile([128, D], BF16, tag="yt")
                    if tt % 2 == 0:
                        nc.vector.tensor_copy(
                            yt[:], psy[:].rearrange("p h i -> p (h i)"))
                    else:
                        nc.scalar.copy(
                            yt[:], psy[:].rearrange("p h i -> p (h i)"))
                    nc.sync.dma_start(
                        y_out.ap()[b, tt * 128:(tt + 1) * 128, :], yt[:])

    nc.compile()
    return nc


_NC_CACHE = {}


def _get_program(has_beta=False):
    if has_beta not in _NC_CACHE:
        _NC_CACHE[has_beta] = build_program(has_beta)
    return _NC_CACHE[has_beta]


def build_in_maps(x, attn_bias, ln_gamma, ln_beta, w_qkv, w_out):
    x = np.asarray(x, dtype=np.float32)
    attn_bias = np.asarray(attn_bias, dtype=np.float32)
    ln_gamma = np.asarray(ln_gamma, dtype=np.float64)
    ln_beta = np.asarray(ln_beta, dtype=np.float64)
    w_qkv = np.asarray(w_qkv, dtype=np.float64)
    w_out = np.asarray(w_out, dtype=np.float32)

    ident = np.eye(128, dtype=ml_dtypes.bfloat16)
    # x pre-sliced per (batch, 512-token slice): xs[b, nt, p, k, i]
    xs = np.ascontiguousarray(
        x.reshape(B, IT, 512, KS, 128).transpose(0, 1, 4, 3, 2)
    ).astype(ml_dtypes.bfloat16).reshape(B, IT, 128, KS * 512)
    # causal mask folded into the bias, transposed to [head, key j, query i]
    tri = np.triu(np.ones((N, N), dtype=bool), k=1)  # True above diag (masked)
    in_maps = []
    for c in range(N_CORES):
        h0 = HL * c
        cols = np.concatenate([
            w_qkv[:, q * H * DH + h0 * DH: q * H * DH + (h0 + HL) * DH]
            for q in range(3)], axis=1)
        # gamma-folded, q-scaled weights (host-side prep)
        colsq = cols.copy()
        colsq[:, 0:128] *= SCALE
        wgam = colsq * ln_gamma[:, None]
        u = wgam.sum(axis=0)            # colsum(W') for the mu rank-1
        bw = ln_beta @ colsq            # beta @ W (raw, q-scaled)
        bwun = np.stack([-u, bw]).astype(ml_dtypes.bfloat16)
        wqkvb = np.ascontiguousarray(
            wgam.reshape(KS, 128, COLS).transpose(1, 0, 2)
        ).astype(ml_dtypes.bfloat16)
        biasT = np.empty((HL, N, N), dtype=ml_dtypes.bfloat16)
        for h in range(HL):
            bh = attn_bias[h0 + h].copy()
            bh[tri] = NEG
            biasT[h] = bh.T.astype(ml_dtypes.bfloat16)
        in_maps.append({
            "xs": xs,
            "biasT": biasT,
            "wqkvb": wqkvb,
            "bwun": bwun,
            "woutb": np.ascontiguousarray(
                w_out[h0 * DH:(h0 + HL) * DH]).astype(ml_dtypes.bfloat16),
            "ident": ident,
        })
    return in_maps


def kernel(x, attn_bias, ln_gamma, ln_beta, w_qkv, w_out):
    in_maps = build_in_maps(x, attn_bias, ln_gamma, ln_beta, w_qkv, w_out)
    nc = _get_program(has_beta=bool(np.any(np.asarray(ln_beta))))
    res = run_bass_kernel_spmd(nc, in_maps, core_ids=list(range(N_CORES)))
    out = res.results[0]["y"].astype(np.float32)
    for c in range(1, N_CORES):
        out += res.results[c]["y"].astype(np.float32)
    return out
